# revision 16
# baseline (speedup 1.0000x reference)
"""Trainium2 Bass kernel for causal GQA attention (B=2, S=2048, D=2048,
H=32, KVH=8, hd=64) with RoPE and output projection, running SPMD on 8
NeuronCores.

Sharding: tensor-parallel over heads (4-way) x data-parallel over batch
(2-way).  Core c (b = c//4, k = c%4) handles batch b and heads
8k..8k+8 (kv heads 2k, 2k+1).  Attention outputs are AllGathered within
each batch group of 4 cores once per q tile; each core then computes a
512-wide output-dim slice of the wo projection.  The host assembles the
full output, so no AllReduce is needed.

All matmul operands are bf16 (fp32 PSUM accumulation): bf16 streams at
1 cyc/row, uses separate LDWEIGHTS instructions that the PE pulls ahead
of in-flight matmuls, and halves DMA + collective bytes.  The K=64
score matmuls for the two heads of a pair sit on disjoint PE row groups
(partitions 0:64 / 64:128, auto tile_position) and different PSUM
banks, so they run concurrently.

Emission interleaves projection s-tiles with attention q-tiles so the
AllGather chain starts ~3 s-tiles earlier and the wo matmuls for tile t
are emitted after attention tile t+1 (their gathered input has landed
by the time the PE queue reaches them).
"""

import numpy as np

DIM = 2048
S = 2048
B = 2
H = 32
KVH = 8
HD = 64
P = 128
HL = 8          # heads per core
QT = 512        # q tile (free dim of score matmuls)
NQT = S // QT   # 4
NKV = S // P    # 16 kv tiles of 128
DK = DIM // P   # 16 contraction tiles
ROPE_BASE = 10000.0
N_CORES = 8

_CACHE = {}


def _build():
    import concourse.bacc as bacc
    import concourse.tile as tile
    import concourse.mybir as mybir
    from concourse.masks import make_identity

    F32 = mybir.dt.float32
    F32R = mybir.dt.float32r
    BF16 = mybir.dt.bfloat16
    Exp = mybir.ActivationFunctionType.Exp

    nc = bacc.Bacc("TRN2", target_bir_lowering=False, debug=False,
                   num_devices=N_CORES)

    xT = nc.dram_tensor("xT", [DIM, S], BF16, kind="ExternalInput").ap()
    wqT = nc.dram_tensor("wqT", [DIM, 512], BF16, kind="ExternalInput").ap()
    wkT = nc.dram_tensor("wkT", [DIM, P], BF16, kind="ExternalInput").ap()
    wvT = nc.dram_tensor("wvT", [DIM, P], BF16, kind="ExternalInput").ap()
    woT = nc.dram_tensor("woT", [DIM, 512], BF16, kind="ExternalInput").ap()
    cosT = nc.dram_tensor("cosT", [P, S], BF16, kind="ExternalInput").ap()
    sinT = nc.dram_tensor("sinT", [P, S], BF16, kind="ExternalInput").ap()
    maskT = nc.dram_tensor("maskT", [P, P], BF16, kind="ExternalInput").ap()
    out_t = nc.dram_tensor("out_t", [512, S], F32, kind="ExternalOutput").ap()

    xT3 = xT.rearrange("(o p) s -> p o s", p=P)
    wqT3 = wqT.rearrange("(o p) f -> p o f", p=P)
    wkT3 = wkT.rearrange("(o p) f -> p o f", p=P)
    wvT3 = wvT.rearrange("(o p) f -> p o f", p=P)
    woT3 = woT.rearrange("(o p) f -> p o f", p=P)

    with tile.TileContext(nc) as tc:
        with (
            tc.tile_pool(name="pers", bufs=1) as pers,
            tc.tile_pool(name="pw", bufs=1) as pw,
            tc.tile_pool(name="ps", bufs=1, space="PSUM") as ps,
            tc.tile_pool(name="dram", bufs=1, space="DRAM") as dram,
        ):
            # ---- persistent tiles ----
            q_fin = [pers.tile([P, S], BF16, name=f"q_fin{m}") for m in range(4)]
            k_fin = [pers.tile([P, S], BF16, name=f"k_fin{g}") for g in range(2)]
            v1 = [pers.tile([P, NKV, P], BF16, name=f"v1_{g}") for g in range(2)]
            # single [P, P] lower-triangle (s >= p) mask: every diagonal
            # 128-chunk's triangular block uses the same pattern
            msk = pers.tile([P, P], BF16, name="msk")
            cos_sb = pers.tile([P, S], BF16, name="cos_sb")
            sin_sb = pers.tile([P, S], BF16, name="sin_sb")
            wq_sb = [pers.tile([P, DK, P], BF16, name=f"wq_sb{m}")
                     for m in range(4)]
            wk_sb = pers.tile([P, DK, P], BF16, name="wk_sb")
            wv_sb = pers.tile([P, DK, P], BF16, name="wv_sb")
            wo_sb = pers.tile([P, DK, 512], BF16, name="wo_sb")
            ident_f = pers.tile([P, P], F32, name="ident_f")
            ident = pers.tile([P, P], F32R, name="ident")

            cc_in = [dram.tile([512, QT], BF16, name=f"cc_in{t}")
                     for t in range(NQT)]
            cc_out = [dram.tile([4 * 512, QT], BF16, name=f"cc_out{t}")
                      for t in range(NQT)]
            cct_tiles = {}

            # PSUM layout (8 banks): tag sc2 = 3 bufs x 2 banks (q proj +
            # scores), tag pv = 2 bufs x 1 bank (k/v proj, PV accum, V
            # transpose, wo accum).
            def sc2(name):
                return ps.tile([P, 2, QT], F32, tag="sc2", bufs=3, name=name)

            def pvb(name, shape=None, dtype=None):
                return ps.tile(shape or [P, QT], dtype or F32, tag="pv",
                               bufs=2, name=name)

            # ---------------- initial DMAs.  Weights go on the scalar
            # engine's DMA queue and wo on the vector engine's queue so
            # they run on different hardware rings than the x chunks
            # (sync queue) and don't head-of-line block them.
            OCH = 2  # contraction 128-tiles per x DMA chunk
            WCH = 4  # contraction 128-tiles per weight DMA chunk
            xsl0 = pw.tile([P, OCH, QT], BF16, tag="xsl", bufs=6,
                           name="xsl0")
            nc.sync.dma_start(xsl0[:], xT3[:, 0:OCH, 0:QT])
            for c in range(DK // WCH):
                osl = slice(c * WCH, (c + 1) * WCH)
                for m in range(4):
                    nc.scalar.dma_start(wq_sb[m][:, osl, :],
                                        wqT3[:, osl, m * P:(m + 1) * P])
                nc.scalar.dma_start(wk_sb[:, osl, :], wkT3[:, osl, :])
                nc.scalar.dma_start(wv_sb[:, osl, :], wvT3[:, osl, :])
            make_identity(nc, ident_f[:])
            nc.vector.tensor_copy(ident[:], ident_f[:])
            for c in range(DK // WCH):
                osl = slice(c * WCH, (c + 1) * WCH)
                nc.gpsimd.dma_start(wo_sb[:, osl, :], woT3[:, osl, :])

            # ones columns of the PV stationary operand (also computes the
            # softmax denominator): memset f32 then cast-copy to bf16
            ones3 = pw.tile([P, NKV, HD], F32, name="ones3")
            nc.vector.memset(ones3[:], 1.0)
            for g in range(2):
                nc.vector.tensor_copy(v1[g][:, :, 0:HD], ones3[:])

            # ================= projection + RoPE for s-tile st ===========
            xpre = {(0, 0): xsl0}

            def prefetch_x(st, oc):
                xsl = pw.tile([P, OCH, QT], BF16, tag="xsl", bufs=6,
                              name="xsl")
                nc.sync.dma_start(
                    xsl[:], xT3[:, oc * OCH:(oc + 1) * OCH,
                                st * QT:(st + 1) * QT])
                xpre[(st, oc)] = xsl

            def proj_tile(st):
                ssl = slice(st * QT, (st + 1) * QT)
                qa = sc2(f"qa{st}")
                qb = sc2(f"qb{st}")
                kk = pvb(f"kk{st}")
                vv = pvb(f"vv{st}")
                qps = [qa[:, 0, :], qa[:, 1, :], qb[:, 0, :], qb[:, 1, :]]
                for oc in range(DK // OCH):
                    if (st, oc) in xpre:
                        xsl = xpre.pop((st, oc))
                    else:
                        xsl = pw.tile([P, OCH, QT], BF16, tag="xsl",
                                      bufs=6, name="xsl")
                        nc.sync.dma_start(
                            xsl[:], xT3[:, oc * OCH:(oc + 1) * OCH, ssl])
                    for oo in range(OCH):
                        o = oc * OCH + oo
                        first = o == 0
                        last = o == DK - 1
                        for m in range(4):
                            nc.tensor.matmul(
                                qps[m], wq_sb[m][:, o, :], xsl[:, oo, :],
                                start=first, stop=last)
                        nc.tensor.matmul(kk[:], wk_sb[:, o, :],
                                         xsl[:, oo, :],
                                         start=first, stop=last)
                        nc.tensor.matmul(vv[:], wv_sb[:, o, :],
                                         xsl[:, oo, :],
                                         start=first, stop=last)

                # RoPE on the 4 q slices and the (2-kv-head) k slice.  The
                # psum->sbuf cast copies are split across DVE and ACT so
                # the psum banks drain in parallel.
                k2 = pw.tile([P, QT], BF16, tag="k2", bufs=2, name="k2")
                for i, (dst, src) in enumerate(
                        [(q_fin[m][:, ssl], qps[m]) for m in range(4)]
                        + [(k2[:], kk[:])]):
                    raw = pw.tile([P, QT], BF16, tag="raw", bufs=4,
                                  name="raw")
                    if i % 2 == 0:
                        nc.vector.tensor_copy(raw[:], src)
                    else:
                        nc.scalar.copy(raw[:], src)
                    rot = pw.tile([P, QT], BF16, tag="rot", bufs=3,
                                  name="rot")
                    for hh in range(2):
                        base = hh * HD
                        nc.sync.dma_start(rot[base:base + 32, :],
                                          raw[base + 32:base + 64, :])
                        nc.sync.dma_start(rot[base + 32:base + 64, :],
                                          raw[base:base + 32, :])
                    nc.vector.tensor_mul(rot[:], rot[:], sin_sb[:, ssl])
                    nc.vector.tensor_mul(raw[:], raw[:], cos_sb[:, ssl])
                    nc.vector.tensor_add(dst, raw[:], rot[:])
                # K dedup: k_fin[g] holds kv head g in BOTH partition
                # halves (the score matmuls for the two heads of a pair
                # need the same kv head at base partitions 0 and 64).
                nc.vector.tensor_copy(k_fin[0][0:HD, ssl], k2[0:HD, :])
                nc.scalar.copy(k_fin[1][HD:P, ssl], k2[HD:P, :])
                nc.sync.dma_start(k_fin[0][HD:P, ssl], k2[0:HD, :])
                nc.sync.dma_start(k_fin[1][0:HD, ssl], k2[HD:P, :])

                # V: drain psum, PE-transpose each 128-chunk, pack into v1
                vts = pw.tile([P, QT], F32R, tag="vts", bufs=2, name="vts")
                nc.scalar.copy(vts[:], vv[:])
                for jj in range(QT // P):
                    j = st * (QT // P) + jj
                    pst = pvb(f"pst{j}", [P, P], F32R)
                    nc.tensor.transpose(pst[:],
                                        vts[:, jj * P:(jj + 1) * P],
                                        ident[:])
                    nc.vector.tensor_copy(v1[0][:, j, HD:P], pst[:, 0:HD])
                    nc.scalar.copy(v1[1][:, j, HD:P], pst[:, HD:P])

            # ================= attention for q tile t ====================
            def attn_pair(t, m):
                """Heads 2m and 2m+1 together on disjoint partition halves
                (0:64 / 64:128) and different psum banks.  The two
                diagonal kv groups are trimmed: kv chunk c of the diagonal
                only needs q >= 128c, so score matmuls, exp, and PV run on
                the shortened q range and only the 128-wide triangular
                block gets masked."""
                ngrp = 2 * (t + 1)
                g = m // 2
                prs = [slice(0, HD), slice(HD, P)]
                pspv = [pvb(f"pv_{t}_{m}_{hf}") for hf in range(2)]
                e_pair = []
                for g2 in range(ngrp):
                    cpair = g2 - 2 * t
                    # local q start per i-chunk (0 for off-diagonal)
                    qs = [P * (2 * cpair + i) if cpair >= 0 else 0
                          for i in range(2)]
                    pss = [sc2(f"ss_{t}_{m}_{g2}_{hf}") for hf in range(2)]
                    for i in range(2):
                        j = 2 * g2 + i
                        for hf in range(2):
                            nc.tensor.matmul(
                                pss[hf][:, i, qs[i]:],
                                k_fin[g][prs[hf], j * P:(j + 1) * P],
                                q_fin[m][prs[hf],
                                         t * QT + qs[i]:(t + 1) * QT],
                                start=True, stop=True)
                    e2 = []
                    for hf in range(2):
                        e = pw.tile([P, 2, QT], BF16, tag="exp", bufs=6,
                                    name="e2")
                        if cpair < 0:
                            nc.scalar.activation(e[:], pss[hf][:], Exp,
                                                 scale=0.125)
                        else:
                            for i in range(2):
                                nc.scalar.activation(
                                    e[:, i, qs[i]:], pss[hf][:, i, qs[i]:],
                                    Exp, scale=0.125)
                                nc.vector.tensor_mul(
                                    e[:, i, qs[i]:qs[i] + P],
                                    e[:, i, qs[i]:qs[i] + P], msk[:])
                        e2.append(e)
                    e_pair.append((e2, qs))
                    if g2 >= 1:
                        ep, qp = e_pair[g2 - 1]
                        for i in range(2):
                            j = 2 * (g2 - 1) + i
                            for hf in range(2):
                                nc.tensor.matmul(
                                    pspv[hf][:, qp[i]:], v1[g][:, j, :],
                                    ep[hf][:, i, qp[i]:],
                                    start=(j == 0), stop=False)
                ep, qp = e_pair[ngrp - 1]
                for i in range(2):
                    j = 2 * (ngrp - 1) + i
                    for hf in range(2):
                        nc.tensor.matmul(
                            pspv[hf][:, qp[i]:], v1[g][:, j, :],
                            ep[hf][:, i, qp[i]:],
                            start=(j == 0), stop=(j == 4 * t + 3))
                for hf in range(2):
                    h = 2 * m + hf
                    # full copy so the pv psum bank releases while the
                    # normalize chain continues from SBUF (on ACT; the
                    # DVE is the loaded engine during attention)
                    ocp = pw.tile([P, QT], F32, tag="ocp", bufs=3,
                                  name="ocp")
                    nc.scalar.copy(ocp[:], pspv[hf][:])
                    # rows 0:64 all hold the denominator L (the 64 ones
                    # columns of v1), so take reciprocals there and move
                    # them to partitions 64:128 with one partition-shift
                    # SBUF-SBUF DMA -- no DRAM bounce needed.
                    recip = pw.tile([P, QT], F32, tag="recip", bufs=2,
                                    name="recip")
                    nc.vector.reciprocal_approx_fast(recip[0:HD, :],
                                                     ocp[0:HD, :])
                    nc.sync.dma_start(recip[HD:P, :], recip[0:HD, :])
                    o_sb = pw.tile([P, QT], BF16, tag="osb", bufs=3,
                                   name="o_sb")
                    nc.vector.tensor_mul(o_sb[HD:P, :], ocp[HD:P, :],
                                         recip[HD:P, :])
                    nc.sync.dma_start(cc_in[t][h * HD:(h + 1) * HD, :],
                                      o_sb[HD:P, :])

            def trig_ag(t):
                nc.gpsimd.collective_compute(
                    "AllGather",
                    mybir.AluOpType.bypass,
                    replica_groups=[[0, 1, 2, 3], [4, 5, 6, 7]],
                    ins=[cc_in[t][:].opt()],
                    outs=[cc_out[t][:].opt()],
                )

            def load_cct(t):
                cct_tiles[t] = pw.tile([P, DK, QT], BF16, tag="cct",
                                       bufs=2, name="cct")
                cc3 = cc_out[t][:].rearrange("(o p) s -> p o s", p=P)
                nc.gpsimd.dma_start(cct_tiles[t][:], cc3[:])

            def wo_tile(t):
                qsl = slice(t * QT, (t + 1) * QT)
                cct = cct_tiles[t]
                for d in range(4):
                    pwm = pvb(f"wo_{t}_{d}")
                    for o in range(DK):
                        nc.tensor.matmul(
                            pwm[:],
                            wo_sb[:, o, d * P:(d + 1) * P],
                            cct[:, o, :],
                            start=(o == 0), stop=(o == DK - 1))
                    ot = pw.tile([P, QT], F32, tag="ot", bufs=2,
                                 name="ot")
                    if d % 2 == 0:
                        nc.vector.tensor_copy(ot[:], pwm[:])
                    else:
                        nc.scalar.copy(ot[:], pwm[:])
                    nc.sync.dma_start(out_t[d * P:(d + 1) * P, qsl],
                                      ot[:])

            # ================= emission schedule =========================
            # PE queue order: proj0 proj1 attn0 proj2 attn1 wo0 proj3
            # attn2 wo1 attn3 wo2 wo3.  wo(t) is emitted one attention
            # tile after AG(t) fires so its gathered input has landed by
            # the time the PE reaches it.  gpsimd runs the (blocking)
            # AllGather chain plus the cct loads only.
            # early x chunks ahead of the cos/sin/mask loads so the sync
            # ring keeps feeding the first projection matmuls
            prefetch_x(0, 1)
            prefetch_x(0, 2)
            prefetch_x(0, 3)
            nc.sync.dma_start(cos_sb[:], cosT[:])
            nc.sync.dma_start(sin_sb[:], sinT[:])
            nc.sync.dma_start(msk[:], maskT[:])
            proj_tile(0)
            proj_tile(1)
            prefetch_x(2, 0)
            prefetch_x(2, 1)
            for m in range(4):
                attn_pair(0, m)
            trig_ag(0)
            load_cct(0)
            proj_tile(2)
            prefetch_x(3, 0)
            prefetch_x(3, 1)
            for m in range(4):
                attn_pair(1, m)
            trig_ag(1)
            load_cct(1)
            wo_tile(0)
            proj_tile(3)
            for m in range(4):
                attn_pair(2, m)
            trig_ag(2)
            load_cct(2)
            wo_tile(1)
            for m in range(4):
                attn_pair(3, m)
            trig_ag(3)
            load_cct(3)
            wo_tile(2)
            wo_tile(3)

    nc.compile()
    return nc


def _prep_inputs(x, position_ids, wq, wk, wv, wo):
    from ml_dtypes import bfloat16

    x = np.asarray(x, dtype=np.float32)
    pos = np.asarray(position_ids).reshape(-1).astype(np.int64)
    wqTf = np.asarray(wq, dtype=np.float32).T
    wkTf = np.asarray(wk, dtype=np.float32).T
    wvTf = np.asarray(wv, dtype=np.float32).T
    woTf = np.asarray(wo, dtype=np.float32).T

    inv = 1.0 / (ROPE_BASE ** (np.arange(0, HD, 2, dtype=np.float32) / HD))
    freqs = np.outer(pos.astype(np.float32), inv)  # [S, 32]
    pidx = np.arange(P) % 32
    sign = np.where((np.arange(P) % HD) < 32, -1.0, 1.0).astype(np.float32)
    cosT = np.ascontiguousarray(np.cos(freqs)[:, pidx].T).astype(bfloat16)
    sinT = np.ascontiguousarray(
        np.sin(freqs)[:, pidx].T * sign[:, None]).astype(bfloat16)

    pg = np.arange(P)[:, None]
    fg = np.arange(P)[None, :]
    maskT = (fg >= pg).astype(bfloat16)

    xT = [np.ascontiguousarray(x[b].T).astype(bfloat16) for b in range(B)]

    in_maps = []
    for c in range(N_CORES):
        b, k = c // 4, c % 4
        in_maps.append({
            "xT": xT[b],
            "wqT": np.ascontiguousarray(
                wqTf[:, 512 * k:512 * (k + 1)]).astype(bfloat16),
            "wkT": np.ascontiguousarray(
                wkTf[:, 128 * k:128 * (k + 1)]).astype(bfloat16),
            "wvT": np.ascontiguousarray(
                wvTf[:, 128 * k:128 * (k + 1)]).astype(bfloat16),
            "woT": np.ascontiguousarray(
                woTf[:, 512 * k:512 * (k + 1)]).astype(bfloat16),
            "cosT": cosT,
            "sinT": sinT,
            "maskT": maskT,
        })
    return in_maps


LAST_EXEC_NS = None


def kernel(x, position_ids, wq, wk, wv, wo, _trace=False):
    import time

    from concourse import bass_utils

    if "nc" not in _CACHE:
        _CACHE["nc"] = _build()
    nc = _CACHE["nc"]

    in_maps = _prep_inputs(x, position_ids, wq, wk, wv, wo)
    res = None
    for attempt in range(3):
        try:
            res = bass_utils.run_bass_kernel_spmd(
                nc, in_maps, core_ids=list(range(N_CORES)), trace=_trace)
            break
        except Exception:
            # transient device hiccups (e.g. NRT_EXEC_UNIT_UNRECOVERABLE
            # after rapid back-to-back runs) usually clear on retry
            if attempt == 2:
                raise
            time.sleep(20 * (attempt + 1))

    global LAST_EXEC_NS
    LAST_EXEC_NS = res.exec_time_ns

    out = np.empty((B, S, DIM), dtype=np.float32)
    for c in range(N_CORES):
        b, k = c // 4, c % 4
        out[b, :, 512 * k:512 * (k + 1)] = res.results[c]["out_t"].T
    return out
